# revision 21
# baseline (speedup 1.0000x reference)
"""Trainium2 Bass kernel for nn_MemoryReader (fp8 DoubleRow mm2).

Reference computation (per batch b):
    mi = mk.reshape(CK, N);  qi = qk.reshape(CK, P) / sqrt(CK)
    S  = mi.T @ qi                      # [N, P] affinity logits
    A  = softmax(S, axis=0)             # over memory axis N
    mem = mv.reshape(CV, N) @ A         # [CV, P]
    out = concat([mem, qv], axis=channel)

Sharding: 8 cores = (4 batches) x (2 halves of the memory axis N).
Each core computes, for its (b, half):
    E      = exp(S_half/8 - ln4)                # constant offset cancels in softmax
    memT   = E.T @ mv_half.T                    # [P, CV] unnormalized numerator
    r2     = per-(partition,plane) partial sums of E   # [128, 2, P]
The host combines: lsum = sum_rows(r2_0 + r2_1), mem = (m0 + m1) / lsum,
then concats qv (pure passthrough). No on-device collectives needed.

Device layout notes (v2, fp8):
  - mm1 (bf16): S for an even/odd n-tile PAIR is written into one 2-bank
    PSUM tile S2[128, 2, 512] (each plane = exactly one bank, so each
    matmul stays within a bank).
  - One exp ACTIVATE per (pair, chunk) converts S2[:, :, :w] -> fp8e4m3
    e8[128, 2, w]: this IS the DoubleRow [Ki, Ko=2, M] stationary layout.
  - mm2 runs perf_mode=DoubleRow: contraction 256 rows/pass, halving PE
    time vs bf16. rhs is host-packed fp8 mvT[128, pair, 2, CV].
  - n is padded 6480 -> 6656 (52 tiles): pad rows have zero mk -> S=0 ->
    E=0.25 (finite), and zero mv rows -> no numerator pollution. The
    denominator adds slice the pads out ([:80] of pair 25 plane 0).
  - Denominator: DVE accumulates r2 += e8 per pair (fp8 read, 1x mode);
    r2 is DMA'd raw and the host does the final 256-row sum. No PE
    l-matmuls, no PSUM bank for them.
  - p-axis chunks of 384 (3 PSUM acc banks) + final 84; PSUM = 2*S2(2) +
    4 acc (3 live per chunk, 4th lets the next chunk start early) = 8.
"""

import math

import numpy as np
import ml_dtypes

import concourse.tile as tile
from concourse import bacc, mybir
from concourse.bass_utils import run_bass_kernel_spmd

# Problem shape (hardcoded per contract)
B, CK, CV, T, H, W = 4, 64, 512, 8, 30, 54
N = T * H * W          # 12960 memory positions
P = H * W              # 1620 query positions
NHALF = N // 2         # 6480 per core
NT = 52                # n-tiles of 128 (padded: 6656; real rows 6480)
NT2 = NT // 2          # 26 DoubleRow pairs
NPAD = NT * 128        # 6656
NLAST = NHALF - 50 * 128    # 80 real rows in tile 50; tile 51 all pad
# p-axis chunking: (ps, width, n_slices); 512 = 4 psum acc banks per chunk
# (wide chunks lengthen the PE pair-window so the exp latency hides).
PCHUNKS = [(0, 512, 4), (512, 512, 4), (1024, 512, 4), (1536, 84, 1)]
EXP_BIAS = -math.log(4.0)  # keeps E in fp8e4m3 range; cancels in softmax

_CACHE = {}


def _build_program():
    bf16 = mybir.dt.bfloat16
    f8 = mybir.dt.float8e4
    f32 = mybir.dt.float32
    nc = bacc.Bacc(None, target_bir_lowering=False, debug=False)

    mk_d = nc.declare_dram_parameter("mk", [128, NT, 128], bf16, isOutput=False)
    qk_d = nc.declare_dram_parameter("qk", [128, P], bf16, isOutput=False)
    mv8_d = nc.declare_dram_parameter("mv8", [128, NT2, 2, CV], f8, isOutput=False)
    # outputs: memT[p, v] (transposed numerator); r2 raw partial sums
    mem_d = nc.declare_dram_parameter("memT", [P, CV], f32, isOutput=True)
    l_d = nc.declare_dram_parameter("lsum", [128, 2, P], f32, isOutput=True)

    with tile.TileContext(nc) as tc:
        with (
            tc.tile_pool(name="singles", bufs=1) as singles,
            tc.tile_pool(name="epool", bufs=6) as epool,
            tc.tile_pool(name="opool", bufs=8) as opool,
            tc.tile_pool(name="rpool", bufs=2) as rpool,
            tc.tile_pool(name="spsum", bufs=2, space="PSUM") as spsum,
            tc.tile_pool(name="accpsum", bufs=4, space="PSUM") as accpsum,
        ):
            qk_sb = singles.tile([128, P], bf16)
            mk_sb = singles.tile([128, NT, 128], bf16)
            mv8_sb = singles.tile([128, NT2, 2, CV], f8)
            # loads in consumption order, smallest-first so pair 0 of
            # chunk 0 is runnable after ~0.33MB of queue traffic
            def ld_qk(c):
                ps_, w_, _ = PCHUNKS[c]
                nc.sync.dma_start(
                    out=qk_sb[:, ps_:ps_ + w_], in_=qk_d[:, ps_:ps_ + w_]
                )

            def ld_mk(a, b):
                nc.sync.dma_start(out=mk_sb[:, a:b, :], in_=mk_d[:, a:b, :])

            def ld_mv(a, b):
                nc.sync.dma_start(
                    out=mv8_sb[:, a:b, :, :], in_=mv8_d[:, a:b, :, :]
                )

            ld_qk(0)
            ld_mk(0, 2)
            ld_mv(0, 1)
            ld_mk(2, 6)
            ld_mv(1, 3)
            ld_qk(1)
            ld_mk(6, 14)
            ld_mv(3, 6)
            ld_qk(2)
            ld_qk(3)
            ld_mk(14, 26)
            ld_mv(6, 10)
            ld_mk(26, NT)
            ld_mv(10, 18)
            ld_mv(18, NT2)

            # Warm-up: matmuls on a memset tile, depending on no DMA. They
            # run while the input DMAs land, filling the initial PE idle
            # gap AND releasing the HAM clock throttle. The memset runs on
            # GPSIMD (its queue is free right after the preamble) so the
            # first warm MM issues ~2.5us earlier than via DVE.
            warmw = singles.tile([128, 256], bf16, name="warmw")
            nc.gpsimd.memset(warmw, 1.0)
            bias_sb = singles.tile([128, 1], f32, name="bias")
            nc.vector.memset(bias_sb, EXP_BIAS)
            warm = accpsum.tile([128, 256], f32, tag="acc", name="warm")
            for _ in range(22):
                nc.tensor.matmul(
                    warm,
                    lhsT=warmw[:, :128],
                    rhs=warmw,
                    start=True,
                    stop=True,
                )

            s_tiles = {}

            def issue_mm1(ci, t):
                ps, w, _ = PCHUNKS[ci]
                s2 = spsum.tile([128, 2, 512], f32, tag="s", name="s")
                # tile 51 (pair 25 plane 1) is all-pad: skip its matmul/exp
                planes = (0,) if t == NT2 - 1 else (0, 1)
                for j in planes:
                    nc.tensor.matmul(
                        s2[:, j, :w],
                        lhsT=mk_sb[:, 2 * t + j, :],
                        rhs=qk_sb[:, ps:ps + w],
                        start=True,
                        stop=True,
                    )
                s_tiles[(ci, t)] = s2

            # one flat software-pipelined stream over (chunk, pair): the
            # next pair's mm1 is always issued one step ahead, ACROSS
            # chunk boundaries, so the exp pipeline never refills
            seq = [(ci, t) for ci in range(len(PCHUNKS)) for t in range(NT2)]
            issue_mm1(*seq[0])
            acc = None
            for i, (ci, t) in enumerate(seq):
                ps, w, nsl = PCHUNKS[ci]
                if i + 1 < len(seq):
                    issue_mm1(*seq[i + 1])
                if t == 0:
                    acc = []
                    for sl in range(nsl):
                        acc.append(
                            accpsum.tile([128, CV], f32, tag="acc", name="acc")
                        )
                    r2 = rpool.tile([128, 2, 512], f32, tag="r", name="r")
                if True:
                    s2 = s_tiles.pop((ci, t))
                    e8 = epool.tile([128, 2, 512], f8, tag="e", name="e")
                    # pair 25 has no plane-1 data; its stale e8 plane-1
                    # contents are finite and multiply zero mv rows
                    jsl = slice(0, 1) if t == NT2 - 1 else slice(0, 2)
                    nc.scalar.activation(
                        out=e8[:, jsl, :w],
                        in_=s2[:, jsl, :w],
                        func=mybir.ActivationFunctionType.Exp,
                        scale=0.125,  # 1/sqrt(CK)
                        bias=bias_sb[:, 0:1],
                    )
                    # denominator partial sums (pads excluded; pair 25 has
                    # only 80 real rows, all in plane 0)
                    if t == 0:
                        nc.vector.tensor_copy(
                            out=r2[:, :, :w], in_=e8[:, :, :w]
                        )
                    elif t < NT2 - 1:
                        nc.vector.tensor_add(
                            out=r2[:, :, :w], in0=r2[:, :, :w], in1=e8[:, :, :w]
                        )
                    else:
                        nc.vector.tensor_add(
                            out=r2[:NLAST, 0, :w],
                            in0=r2[:NLAST, 0, :w],
                            in1=e8[:NLAST, 0, :w],
                        )
                    first, last = t == 0, t == NT2 - 1
                    for sl in range(nsl):
                        pw = min(128, w - sl * 128)
                        nc.tensor.matmul(
                            acc[sl][:pw],
                            lhsT=e8[:, :, sl * 128:sl * 128 + pw],
                            rhs=mv8_sb[:, t, :, :],
                            start=first,
                            stop=last,
                            perf_mode=mybir.MatmulPerfMode.DoubleRow,
                        )

                if t == NT2 - 1:
                    # raw denominator out; host does the 256-row sum
                    nc.sync.dma_start(
                        out=l_d[:, :, ps:ps + w], in_=r2[:, :, :w]
                    )
                    last_chunk = ci == len(PCHUNKS) - 1
                    for sl in range(nsl):
                        pw = min(128, w - sl * 128)
                        o_sb = opool.tile([128, CV], f32, tag="o", name="o")
                        p0 = ps + sl * 128
                        if last_chunk:
                            # end-of-kernel latency path: split the copy
                            # across both engines and the DMA across both
                            # HWDGE queues
                            nc.vector.tensor_copy(
                                out=o_sb[:pw, :256], in_=acc[sl][:pw, :256]
                            )
                            nc.scalar.copy(
                                out=o_sb[:pw, 256:], in_=acc[sl][:pw, 256:]
                            )
                            nc.scalar.dma_start(
                                out=mem_d[p0:p0 + pw, :256],
                                in_=o_sb[:pw, :256],
                            )
                            nc.sync.dma_start(
                                out=mem_d[p0:p0 + pw, 256:],
                                in_=o_sb[:pw, 256:],
                            )
                        else:
                            nc.any.tensor_copy(
                                out=o_sb[:pw], in_=acc[sl][:pw]
                            )
                            nc.sync.dma_start(
                                out=mem_d[p0:p0 + pw, :], in_=o_sb[:pw]
                            )

    _strip_same_engine_waits(nc)
    nc.compile()
    return nc


def _strip_same_engine_waits(nc):
    """Drop redundant same-engine semaphore waits on ACT/PE compute
    instructions (see baseline notes: TRN2 instructions hold one wait;
    extra waits force serializing EventSemaphore instructions)."""
    prefixes = {
        "EngineType.Activation": "Activation_",
        "EngineType.PE": "PE_",
    }
    kinds = (mybir.InstActivation, mybir.InstMatmult, mybir.InstLdweights)
    for fn in nc.m.functions:
        for blk in fn.blocks:
            for inst in blk.instructions:
                si = getattr(inst, "sync_info", None)
                if si is None or not si.on_wait or not isinstance(inst, kinds):
                    continue
                pref = prefixes.get(str(getattr(inst, "engine", None)))
                if pref is None:
                    continue
                kept = [w for w in si.on_wait
                        if not str(getattr(w, "ant_name", "")).startswith(pref)]
                if len(kept) != len(si.on_wait):
                    si.on_wait = kept


def _get_program():
    if "nc" not in _CACHE:
        _CACHE["nc"] = _build_program()
    return _CACHE["nc"]


def _make_in_maps(mk, mv, qk):
    f8 = ml_dtypes.float8_e4m3
    mkf = np.ascontiguousarray(mk.reshape(B, CK, N))
    mvf = np.ascontiguousarray(mv.reshape(B, CV, N))
    qkf = np.ascontiguousarray(qk.reshape(B, CK, P))
    in_maps = []
    for core in range(8):
        b, half = core // 2, core % 2
        n0, n1 = half * NHALF, (half + 1) * NHALF
        mk_c = mkf[b, :, n0:n1].astype(ml_dtypes.bfloat16)   # [64, 6480]
        # zero-pad contraction dim to 128 and n to NPAD
        mk_t = np.zeros((128, NT, 128), dtype=ml_dtypes.bfloat16)
        mk_t[:CK].reshape(CK, NPAD)[:, :NHALF] = mk_c
        qk_c = np.zeros((128, P), dtype=ml_dtypes.bfloat16)
        qk_c[:CK] = qkf[b].astype(ml_dtypes.bfloat16)
        mvt = np.zeros((NPAD, CV), dtype=f8)
        mvt[:NHALF] = mvf[b, :, n0:n1].T.astype(f8)
        # DoubleRow pair layout: [128, pair, plane, CV],
        # elem (k, t, j, v) = mvT[(2t+j)*128 + k, v]
        mv8 = np.ascontiguousarray(
            mvt.reshape(NT2, 2, 128, CV).transpose(2, 0, 1, 3)
        )
        in_maps.append({"mk": np.ascontiguousarray(mk_t),
                        "qk": np.ascontiguousarray(qk_c),
                        "mv8": mv8})
    return in_maps


def _run(mk, mv, qk, qv, trace=False, **spmd_kwargs):
    nc = _get_program()
    in_maps = _make_in_maps(mk, mv, qk)
    res = run_bass_kernel_spmd(nc, in_maps, list(range(8)), trace=trace, **spmd_kwargs)
    out = np.empty((B, 2 * CV, P), dtype=np.float32)
    for b in range(B):
        m0, l0 = res.results[2 * b]["memT"], res.results[2 * b]["lsum"]
        m1, l1 = res.results[2 * b + 1]["memT"], res.results[2 * b + 1]["lsum"]
        lv = (l0 + l1).sum(axis=(0, 1))          # [P]
        out[b, :CV] = ((m0 + m1) / lv[:, None]).T
        out[b, CV:] = qv[b].reshape(CV, P)
    return out.reshape(B, 2 * CV, H, W), res


def kernel(mk, mv, qk, qv):
    out, _ = _run(np.asarray(mk), np.asarray(mv), np.asarray(qk), np.asarray(qv))
    return out


# revision 25
# speedup vs baseline: 1.0020x; 1.0020x over previous
"""Trainium2 Bass kernel for nn_MemoryReader (fp8 DoubleRow mm2).

Reference computation (per batch b):
    mi = mk.reshape(CK, N);  qi = qk.reshape(CK, P) / sqrt(CK)
    S  = mi.T @ qi                      # [N, P] affinity logits
    A  = softmax(S, axis=0)             # over memory axis N
    mem = mv.reshape(CV, N) @ A         # [CV, P]
    out = concat([mem, qv], axis=channel)

Sharding: 8 cores = (4 batches) x (2 halves of the memory axis N).
Each core computes, for its (b, half):
    E      = exp(S_half/8 - ln4)                # constant offset cancels in softmax
    memT   = E.T @ mv_half.T                    # [P, CV] unnormalized numerator
    r2     = per-(partition,plane) partial sums of E   # [128, 2, P]
The host combines: lsum = sum_rows(r2_0 + r2_1), mem = (m0 + m1) / lsum,
then concats qv (pure passthrough). No on-device collectives needed.

Device layout notes (v2, fp8):
  - mm1 (bf16): S for an even/odd n-tile PAIR is written into one 2-bank
    PSUM tile S2[128, 2, 512] (each plane = exactly one bank, so each
    matmul stays within a bank).
  - One exp ACTIVATE per (pair, chunk) converts S2[:, :, :w] -> fp8e4m3
    e8[128, 2, w]: this IS the DoubleRow [Ki, Ko=2, M] stationary layout.
  - mm2 runs perf_mode=DoubleRow: contraction 256 rows/pass, halving PE
    time vs bf16. rhs is host-packed fp8 mvT[128, pair, 2, CV].
  - n is padded 6480 -> 6656 (52 tiles): pad rows have zero mk -> S=0 ->
    E=0.25 (finite), and zero mv rows -> no numerator pollution. The
    denominator adds slice the pads out ([:80] of pair 25 plane 0).
  - Denominator: DVE accumulates r2 += e8 per pair (fp8 read, 1x mode);
    r2 is DMA'd raw and the host does the final 256-row sum. No PE
    l-matmuls, no PSUM bank for them.
  - p-axis chunks of 384 (3 PSUM acc banks) + final 84; PSUM = 2*S2(2) +
    4 acc (3 live per chunk, 4th lets the next chunk start early) = 8.
"""

import math

import numpy as np
import ml_dtypes

import concourse.tile as tile
from concourse import bacc, mybir
from concourse.bass_utils import run_bass_kernel_spmd

# Problem shape (hardcoded per contract)
B, CK, CV, T, H, W = 4, 64, 512, 8, 30, 54
N = T * H * W          # 12960 memory positions
P = H * W              # 1620 query positions
NHALF = N // 2         # 6480 per core
NT = 52                # n-tiles of 128 (padded: 6656; real rows 6480)
NT2 = NT // 2          # 26 DoubleRow pairs
NPAD = NT * 128        # 6656
NLAST = NHALF - 50 * 128    # 80 real rows in tile 50; tile 51 all pad
# p-axis chunking: (ps, width, n_slices); 512 = 4 psum acc banks per chunk
# (wide chunks lengthen the PE pair-window so the exp latency hides).
PCHUNKS = [(0, 512, 4), (512, 512, 4), (1024, 512, 4), (1536, 84, 1)]
EXP_BIAS = -math.log(4.0)  # keeps E in fp8e4m3 range; cancels in softmax

_CACHE = {}


def _build_program():
    bf16 = mybir.dt.bfloat16
    f8 = mybir.dt.float8e4
    f32 = mybir.dt.float32
    nc = bacc.Bacc(None, target_bir_lowering=False, debug=False)

    mk_d = nc.declare_dram_parameter("mk", [128, NT, 128], bf16, isOutput=False)
    qk_d = nc.declare_dram_parameter("qk", [128, P], bf16, isOutput=False)
    mv8_d = nc.declare_dram_parameter("mv8", [128, NT2, 2, CV], f8, isOutput=False)
    # outputs: memT[p, v] (transposed numerator); r2 raw partial sums
    mem_d = nc.declare_dram_parameter("memT", [P, CV], f32, isOutput=True)
    l_d = nc.declare_dram_parameter("lsum", [128, 2, P], f32, isOutput=True)

    with tile.TileContext(nc) as tc:
        with (
            tc.tile_pool(name="singles", bufs=1) as singles,
            tc.tile_pool(name="epool", bufs=6) as epool,
            tc.tile_pool(name="opool", bufs=8) as opool,
            tc.tile_pool(name="rpool", bufs=2) as rpool,
            tc.tile_pool(name="spsum", bufs=2, space="PSUM") as spsum,
            tc.tile_pool(name="accpsum", bufs=4, space="PSUM") as accpsum,
        ):
            qk_sb = singles.tile([128, P], bf16)
            mk_sb = singles.tile([128, NT, 128], bf16)
            mv8_sb = singles.tile([128, NT2, 2, CV], f8)
            # loads in consumption order, smallest-first so pair 0 of
            # chunk 0 is runnable after ~0.33MB of queue traffic
            def ld_qk(c):
                ps_, w_, _ = PCHUNKS[c]
                nc.sync.dma_start(
                    out=qk_sb[:, ps_:ps_ + w_], in_=qk_d[:, ps_:ps_ + w_]
                )

            def ld_mk(a, b):
                nc.sync.dma_start(out=mk_sb[:, a:b, :], in_=mk_d[:, a:b, :])

            def ld_mv(a, b):
                nc.sync.dma_start(
                    out=mv8_sb[:, a:b, :, :], in_=mv8_d[:, a:b, :, :]
                )

            ld_qk(0)
            ld_mk(0, 2)
            ld_mv(0, 1)
            ld_mk(2, 6)
            ld_mv(1, 3)
            ld_qk(1)
            ld_mk(6, 14)
            ld_mv(3, 6)
            ld_qk(2)
            ld_qk(3)
            ld_mk(14, 26)
            ld_mv(6, 10)
            ld_mk(26, NT)
            ld_mv(10, 18)
            ld_mv(18, NT2)

            # Warm-up: matmuls on a memset tile, depending on no DMA. They
            # run while the input DMAs land, filling the initial PE idle
            # gap AND releasing the HAM clock throttle. The memset runs on
            # GPSIMD (its queue is free right after the preamble) so the
            # first warm MM issues ~2.5us earlier than via DVE.
            warmw = singles.tile([128, 256], bf16, name="warmw")
            nc.gpsimd.memset(warmw, 1.0)
            bias_sb = singles.tile([128, 1], f32, name="bias")
            nc.vector.memset(bias_sb, EXP_BIAS)
            warm = accpsum.tile([128, 256], f32, tag="acc", name="warm")
            for _ in range(14):
                nc.tensor.matmul(
                    warm,
                    lhsT=warmw[:, :128],
                    rhs=warmw,
                    start=True,
                    stop=True,
                )

            s_tiles = {}
            LASTC = len(PCHUNKS) - 1  # small 84-wide chunk: 4 tiles/step

            def issue_mm1(ci, t):
                ps, w, _ = PCHUNKS[ci]
                if ci == LASTC:
                    # quad step: tiles 4t..4t+3 share one 1-bank psum tile
                    s2 = spsum.tile([128, 4, 96], f32, tag="s", name="s")
                    planes = (0, 1, 2) if t == NT2 // 2 - 1 else (0, 1, 2, 3)
                else:
                    s2 = spsum.tile([128, 2, 512], f32, tag="s", name="s")
                    # tile 51 (pair 25 plane 1) is all-pad: skip it
                    planes = (0,) if t == NT2 - 1 else (0, 1)
                for j in planes:
                    nc.tensor.matmul(
                        s2[:, j, :w],
                        lhsT=mk_sb[:, (4 if ci == LASTC else 2) * t + j, :],
                        rhs=qk_sb[:, ps:ps + w],
                        start=True,
                        stop=True,
                    )
                s_tiles[(ci, t)] = s2

            # one flat software-pipelined stream over (chunk, step): the
            # next step's mm1 is always issued one step ahead, ACROSS
            # chunk boundaries, so the exp pipeline never refills
            seq = [(ci, t) for ci in range(LASTC) for t in range(NT2)]
            seq += [(LASTC, q) for q in range(NT2 // 2)]
            issue_mm1(*seq[0])
            acc = None
            for i, (ci, t) in enumerate(seq):
                ps, w, nsl = PCHUNKS[ci]
                if i + 1 < len(seq):
                    issue_mm1(*seq[i + 1])
                if t == 0:
                    acc = []
                    for sl in range(nsl):
                        acc.append(
                            accpsum.tile([128, CV], f32, tag="acc", name="acc")
                        )
                    r2 = rpool.tile([128, 2, 512], f32, tag="r", name="r")
                s2 = s_tiles.pop((ci, t))
                if ci == LASTC:
                    # quad step: tiles 4t..4t+3 (pairs 2t, 2t+1) in one go
                    lastq = t == NT2 // 2 - 1
                    e8 = epool.tile([128, 4, 96], f8, tag="e", name="e")
                    jsl = slice(0, 3) if lastq else slice(0, 4)
                    nc.scalar.activation(
                        out=e8[:, jsl, :w],
                        in_=s2[:, jsl, :w],
                        func=mybir.ActivationFunctionType.Exp,
                        scale=0.125,
                        bias=bias_sb[:, 0:1],
                    )
                    if t == 0:
                        nc.vector.tensor_copy(
                            out=r2[:, :, :w], in_=e8[:, 0:2, :w]
                        )
                    else:
                        nc.vector.tensor_add(
                            out=r2[:, :, :w], in0=r2[:, :, :w],
                            in1=e8[:, 0:2, :w],
                        )
                    if lastq:
                        nc.vector.tensor_add(
                            out=r2[:NLAST, 0, :w],
                            in0=r2[:NLAST, 0, :w],
                            in1=e8[:NLAST, 2, :w],
                        )
                    else:
                        nc.vector.tensor_add(
                            out=r2[:, :, :w], in0=r2[:, :, :w],
                            in1=e8[:, 2:4, :w],
                        )
                    pw = w
                    nc.tensor.matmul(
                        acc[0][:pw],
                        lhsT=e8[:, 0:2, :pw],
                        rhs=mv8_sb[:, 2 * t, :, :],
                        start=t == 0,
                        stop=False,
                        perf_mode=mybir.MatmulPerfMode.DoubleRow,
                    )
                    nc.tensor.matmul(
                        acc[0][:pw],
                        lhsT=e8[:, 2:4, :pw],
                        rhs=mv8_sb[:, 2 * t + 1, :, :],
                        start=False,
                        stop=lastq,
                        perf_mode=mybir.MatmulPerfMode.DoubleRow,
                    )
                else:
                    e8 = epool.tile([128, 2, 512], f8, tag="e", name="e")
                    # pair 25 has no plane-1 data; its stale e8 plane-1
                    # contents are finite and multiply zero mv rows
                    jsl = slice(0, 1) if t == NT2 - 1 else slice(0, 2)
                    nc.scalar.activation(
                        out=e8[:, jsl, :w],
                        in_=s2[:, jsl, :w],
                        func=mybir.ActivationFunctionType.Exp,
                        scale=0.125,  # 1/sqrt(CK)
                        bias=bias_sb[:, 0:1],
                    )
                    # denominator partial sums (pads excluded; pair 25 has
                    # only 80 real rows, all in plane 0)
                    if t == 0:
                        nc.vector.tensor_copy(
                            out=r2[:, :, :w], in_=e8[:, :, :w]
                        )
                    elif t < NT2 - 1:
                        nc.vector.tensor_add(
                            out=r2[:, :, :w], in0=r2[:, :, :w], in1=e8[:, :, :w]
                        )
                    else:
                        nc.vector.tensor_add(
                            out=r2[:NLAST, 0, :w],
                            in0=r2[:NLAST, 0, :w],
                            in1=e8[:NLAST, 0, :w],
                        )
                    first, last = t == 0, t == NT2 - 1
                    for sl in range(nsl):
                        pw = min(128, w - sl * 128)
                        nc.tensor.matmul(
                            acc[sl][:pw],
                            lhsT=e8[:, :, sl * 128:sl * 128 + pw],
                            rhs=mv8_sb[:, t, :, :],
                            start=first,
                            stop=last,
                            perf_mode=mybir.MatmulPerfMode.DoubleRow,
                        )

                if t == (NT2 // 2 - 1 if ci == LASTC else NT2 - 1):
                    # raw denominator out; host does the 256-row sum
                    nc.sync.dma_start(
                        out=l_d[:, :, ps:ps + w], in_=r2[:, :, :w]
                    )
                    last_chunk = ci == len(PCHUNKS) - 1
                    for sl in range(nsl):
                        pw = min(128, w - sl * 128)
                        o_sb = opool.tile([128, CV], f32, tag="o", name="o")
                        p0 = ps + sl * 128
                        if last_chunk:
                            # end-of-kernel latency path: split the copy
                            # across both engines and the DMA across both
                            # HWDGE queues
                            nc.vector.tensor_copy(
                                out=o_sb[:pw, :256], in_=acc[sl][:pw, :256]
                            )
                            nc.scalar.copy(
                                out=o_sb[:pw, 256:], in_=acc[sl][:pw, 256:]
                            )
                            nc.scalar.dma_start(
                                out=mem_d[p0:p0 + pw, :256],
                                in_=o_sb[:pw, :256],
                            )
                            nc.sync.dma_start(
                                out=mem_d[p0:p0 + pw, 256:],
                                in_=o_sb[:pw, 256:],
                            )
                        else:
                            nc.any.tensor_copy(
                                out=o_sb[:pw], in_=acc[sl][:pw]
                            )
                            nc.sync.dma_start(
                                out=mem_d[p0:p0 + pw, :], in_=o_sb[:pw]
                            )

    _strip_same_engine_waits(nc)
    nc.compile()
    return nc


def _strip_same_engine_waits(nc):
    """Drop redundant same-engine semaphore waits on ACT/PE compute
    instructions (see baseline notes: TRN2 instructions hold one wait;
    extra waits force serializing EventSemaphore instructions)."""
    prefixes = {
        "EngineType.Activation": "Activation_",
        "EngineType.PE": "PE_",
    }
    kinds = (mybir.InstActivation, mybir.InstMatmult, mybir.InstLdweights)
    for fn in nc.m.functions:
        for blk in fn.blocks:
            for inst in blk.instructions:
                si = getattr(inst, "sync_info", None)
                if si is None or not si.on_wait or not isinstance(inst, kinds):
                    continue
                pref = prefixes.get(str(getattr(inst, "engine", None)))
                if pref is None:
                    continue
                kept = [w for w in si.on_wait
                        if not str(getattr(w, "ant_name", "")).startswith(pref)]
                if len(kept) != len(si.on_wait):
                    si.on_wait = kept


def _get_program():
    if "nc" not in _CACHE:
        _CACHE["nc"] = _build_program()
    return _CACHE["nc"]


def _make_in_maps(mk, mv, qk):
    f8 = ml_dtypes.float8_e4m3
    mkf = np.ascontiguousarray(mk.reshape(B, CK, N))
    mvf = np.ascontiguousarray(mv.reshape(B, CV, N))
    qkf = np.ascontiguousarray(qk.reshape(B, CK, P))
    in_maps = []
    for core in range(8):
        b, half = core // 2, core % 2
        n0, n1 = half * NHALF, (half + 1) * NHALF
        mk_c = mkf[b, :, n0:n1].astype(ml_dtypes.bfloat16)   # [64, 6480]
        # zero-pad contraction dim to 128 and n to NPAD
        mk_t = np.zeros((128, NT, 128), dtype=ml_dtypes.bfloat16)
        mk_t[:CK].reshape(CK, NPAD)[:, :NHALF] = mk_c
        qk_c = np.zeros((128, P), dtype=ml_dtypes.bfloat16)
        qk_c[:CK] = qkf[b].astype(ml_dtypes.bfloat16)
        mvt = np.zeros((NPAD, CV), dtype=f8)
        mvt[:NHALF] = mvf[b, :, n0:n1].T.astype(f8)
        # DoubleRow pair layout: [128, pair, plane, CV],
        # elem (k, t, j, v) = mvT[(2t+j)*128 + k, v]
        mv8 = np.ascontiguousarray(
            mvt.reshape(NT2, 2, 128, CV).transpose(2, 0, 1, 3)
        )
        in_maps.append({"mk": np.ascontiguousarray(mk_t),
                        "qk": np.ascontiguousarray(qk_c),
                        "mv8": mv8})
    return in_maps


def _run(mk, mv, qk, qv, trace=False, **spmd_kwargs):
    nc = _get_program()
    in_maps = _make_in_maps(mk, mv, qk)
    res = run_bass_kernel_spmd(nc, in_maps, list(range(8)), trace=trace, **spmd_kwargs)
    out = np.empty((B, 2 * CV, P), dtype=np.float32)
    for b in range(B):
        m0, l0 = res.results[2 * b]["memT"], res.results[2 * b]["lsum"]
        m1, l1 = res.results[2 * b + 1]["memT"], res.results[2 * b + 1]["lsum"]
        lv = (l0 + l1).sum(axis=(0, 1))          # [P]
        out[b, :CV] = ((m0 + m1) / lv[:, None]).T
        out[b, CV:] = qv[b].reshape(CV, P)
    return out.reshape(B, 2 * CV, H, W), res


def kernel(mk, mv, qk, qv):
    out, _ = _run(np.asarray(mk), np.asarray(mv), np.asarray(qk), np.asarray(qv))
    return out


# revision 29
# speedup vs baseline: 1.0067x; 1.0047x over previous
"""Trainium2 Bass kernel for nn_MemoryReader (fp8 DoubleRow mm2).

Reference computation (per batch b):
    mi = mk.reshape(CK, N);  qi = qk.reshape(CK, P) / sqrt(CK)
    S  = mi.T @ qi                      # [N, P] affinity logits
    A  = softmax(S, axis=0)             # over memory axis N
    mem = mv.reshape(CV, N) @ A         # [CV, P]
    out = concat([mem, qv], axis=channel)

Sharding: 8 cores = (4 batches) x (2 halves of the memory axis N).
Each core computes, for its (b, half):
    E      = exp(S_half/8 - ln4)                # constant offset cancels in softmax
    memT   = E.T @ mv_half.T                    # [P, CV] unnormalized numerator
    r2     = per-(partition,plane) partial sums of E   # [128, 2, P]
The host combines: lsum = sum_rows(r2_0 + r2_1), mem = (m0 + m1) / lsum,
then concats qv (pure passthrough). No on-device collectives needed.

Device layout notes (v2, fp8):
  - mm1 (bf16): S for an even/odd n-tile PAIR is written into one 2-bank
    PSUM tile S2[128, 2, 512] (each plane = exactly one bank, so each
    matmul stays within a bank).
  - One exp ACTIVATE per (pair, chunk) converts S2[:, :, :w] -> fp8e4m3
    e8[128, 2, w]: this IS the DoubleRow [Ki, Ko=2, M] stationary layout.
  - mm2 runs perf_mode=DoubleRow: contraction 256 rows/pass, halving PE
    time vs bf16. rhs is host-packed fp8 mvT[128, pair, 2, CV].
  - n is padded 6480 -> 6656 (52 tiles): pad rows have zero mk -> S=0 ->
    E=0.25 (finite), and zero mv rows -> no numerator pollution. The
    denominator adds slice the pads out ([:80] of pair 25 plane 0).
  - Denominator: DVE accumulates r2 += e8 per pair (fp8 read, 1x mode);
    r2 is DMA'd raw and the host does the final 256-row sum. No PE
    l-matmuls, no PSUM bank for them.
  - p-axis chunks of 512 (4 PSUM acc banks) + final 84 processed as
    4-tile quads; PSUM = 2*S2(2 banks each) + 4 acc = 8 banks.
  - flat software-pipelined (chunk, step) stream with one-step mm1
    lookahead across chunk boundaries; output copies split in halves so
    the next chunk's exp interleaves with them on ACT/DVE.
"""

import math

import numpy as np
import ml_dtypes

import concourse.tile as tile
from concourse import bacc, mybir
from concourse.bass_utils import run_bass_kernel_spmd

# Problem shape (hardcoded per contract)
B, CK, CV, T, H, W = 4, 64, 512, 8, 30, 54
N = T * H * W          # 12960 memory positions
P = H * W              # 1620 query positions
NHALF = N // 2         # 6480 per core
NT = 52                # n-tiles of 128 (padded: 6656; real rows 6480)
NT2 = NT // 2          # 26 DoubleRow pairs
NPAD = NT * 128        # 6656
NLAST = NHALF - 50 * 128    # 80 real rows in tile 50; tile 51 all pad
# p-axis chunking: (ps, width, n_slices); 512 = 4 psum acc banks per chunk
# (wide chunks lengthen the PE pair-window so the exp latency hides).
PCHUNKS = [(0, 512, 4), (512, 512, 4), (1024, 512, 4), (1536, 84, 1)]
EXP_BIAS = -math.log(4.0)  # keeps E in fp8e4m3 range; cancels in softmax

_CACHE = {}


def _build_program():
    bf16 = mybir.dt.bfloat16
    f8 = mybir.dt.float8e4
    f32 = mybir.dt.float32
    nc = bacc.Bacc(None, target_bir_lowering=False, debug=False)

    mk_d = nc.declare_dram_parameter("mk", [128, NT, 128], bf16, isOutput=False)
    qk_d = nc.declare_dram_parameter("qk", [128, P], bf16, isOutput=False)
    mv8_d = nc.declare_dram_parameter("mv8", [128, NT2, 2, CV], f8, isOutput=False)
    # outputs: memT[p, v] (transposed numerator); r2 raw partial sums
    mem_d = nc.declare_dram_parameter("memT", [P, CV], f32, isOutput=True)
    l_d = nc.declare_dram_parameter("lsum", [128, 2, P], f32, isOutput=True)

    with tile.TileContext(nc) as tc:
        with (
            tc.tile_pool(name="singles", bufs=1) as singles,
            tc.tile_pool(name="epool", bufs=6) as epool,
            tc.tile_pool(name="opool", bufs=8) as opool,
            tc.tile_pool(name="rpool", bufs=2) as rpool,
            tc.tile_pool(name="spsum", bufs=2, space="PSUM") as spsum,
            tc.tile_pool(name="accpsum", bufs=4, space="PSUM") as accpsum,
        ):
            qk_sb = singles.tile([128, P], bf16)
            mk_sb = singles.tile([128, NT, 128], bf16)
            mv8_sb = singles.tile([128, NT2, 2, CV], f8)
            # loads in consumption order, smallest-first so pair 0 of
            # chunk 0 is runnable after ~0.33MB of queue traffic
            def ld_qk(c):
                ps_, w_, _ = PCHUNKS[c]
                nc.sync.dma_start(
                    out=qk_sb[:, ps_:ps_ + w_], in_=qk_d[:, ps_:ps_ + w_]
                )

            def ld_mk(a, b):
                nc.sync.dma_start(out=mk_sb[:, a:b, :], in_=mk_d[:, a:b, :])

            def ld_mv(a, b):
                nc.sync.dma_start(
                    out=mv8_sb[:, a:b, :, :], in_=mv8_d[:, a:b, :, :]
                )

            ld_qk(0)
            ld_mk(0, 2)
            ld_mv(0, 1)
            ld_mk(2, 6)
            ld_mv(1, 3)
            ld_qk(1)
            ld_mk(6, 14)
            ld_mv(3, 6)
            ld_qk(2)
            ld_qk(3)
            ld_mk(14, 26)
            ld_mv(6, 10)
            ld_mk(26, NT)
            ld_mv(10, 18)
            ld_mv(18, NT2)

            # Warm-up: matmuls on a memset tile, depending on no DMA. They
            # run while the input DMAs land, filling the initial PE idle
            # gap AND releasing the HAM clock throttle. The memset runs on
            # GPSIMD (its queue is free right after the preamble) so the
            # first warm MM issues ~2.5us earlier than via DVE.
            warmw = singles.tile([128, 256], bf16, name="warmw")
            nc.gpsimd.memset(warmw, 1.0)
            bias_sb = singles.tile([128, 1], f32, name="bias")
            nc.vector.memset(bias_sb, EXP_BIAS)
            warm = accpsum.tile([128, 256], f32, tag="acc", name="warm")
            for _ in range(14):
                nc.tensor.matmul(
                    warm,
                    lhsT=warmw[:, :128],
                    rhs=warmw,
                    start=True,
                    stop=True,
                )

            s_tiles = {}
            LASTC = len(PCHUNKS) - 1  # small 84-wide chunk: 4 tiles/step

            def issue_mm1(ci, t):
                ps, w, _ = PCHUNKS[ci]
                if ci == LASTC:
                    # quad step: tiles 4t..4t+3 share one 1-bank psum tile
                    s2 = spsum.tile([128, 4, 96], f32, tag="s", name="s")
                    planes = (0, 1, 2) if t == NT2 // 2 - 1 else (0, 1, 2, 3)
                else:
                    s2 = spsum.tile([128, 2, 512], f32, tag="s", name="s")
                    # tile 51 (pair 25 plane 1) is all-pad: skip it
                    planes = (0,) if t == NT2 - 1 else (0, 1)
                for j in planes:
                    nc.tensor.matmul(
                        s2[:, j, :w],
                        lhsT=mk_sb[:, (4 if ci == LASTC else 2) * t + j, :],
                        rhs=qk_sb[:, ps:ps + w],
                        start=True,
                        stop=True,
                    )
                s_tiles[(ci, t)] = s2

            # one flat software-pipelined stream over (chunk, step): the
            # next step's mm1 is always issued one step ahead, ACROSS
            # chunk boundaries, so the exp pipeline never refills
            seq = [(ci, t) for ci in range(LASTC) for t in range(NT2)]
            seq += [(LASTC, q) for q in range(NT2 // 2)]
            issue_mm1(*seq[0])
            acc = None
            for i, (ci, t) in enumerate(seq):
                ps, w, nsl = PCHUNKS[ci]
                if i + 1 < len(seq):
                    issue_mm1(*seq[i + 1])
                if t == 0:
                    acc = []
                    for sl in range(nsl):
                        acc.append(
                            accpsum.tile([128, CV], f32, tag="acc", name="acc")
                        )
                    r2 = rpool.tile([128, 2, 512], f32, tag="r", name="r")
                s2 = s_tiles.pop((ci, t))
                if ci == LASTC:
                    # quad step: tiles 4t..4t+3 (pairs 2t, 2t+1) in one go
                    lastq = t == NT2 // 2 - 1
                    e8 = epool.tile([128, 4, 96], f8, tag="e", name="e")
                    jsl = slice(0, 3) if lastq else slice(0, 4)
                    nc.scalar.activation(
                        out=e8[:, jsl, :w],
                        in_=s2[:, jsl, :w],
                        func=mybir.ActivationFunctionType.Exp,
                        scale=0.125,
                        bias=bias_sb[:, 0:1],
                    )
                    if t == 0:
                        nc.vector.tensor_copy(
                            out=r2[:, :, :w], in_=e8[:, 0:2, :w]
                        )
                    else:
                        nc.vector.tensor_add(
                            out=r2[:, :, :w], in0=r2[:, :, :w],
                            in1=e8[:, 0:2, :w],
                        )
                    if lastq:
                        nc.vector.tensor_add(
                            out=r2[:NLAST, 0, :w],
                            in0=r2[:NLAST, 0, :w],
                            in1=e8[:NLAST, 2, :w],
                        )
                    else:
                        nc.vector.tensor_add(
                            out=r2[:, :, :w], in0=r2[:, :, :w],
                            in1=e8[:, 2:4, :w],
                        )
                    pw = w
                    nc.tensor.matmul(
                        acc[0][:pw],
                        lhsT=e8[:, 0:2, :pw],
                        rhs=mv8_sb[:, 2 * t, :, :],
                        start=t == 0,
                        stop=False,
                        perf_mode=mybir.MatmulPerfMode.DoubleRow,
                    )
                    nc.tensor.matmul(
                        acc[0][:pw],
                        lhsT=e8[:, 2:4, :pw],
                        rhs=mv8_sb[:, 2 * t + 1, :, :],
                        start=False,
                        stop=lastq,
                        perf_mode=mybir.MatmulPerfMode.DoubleRow,
                    )
                else:
                    e8 = epool.tile([128, 2, 512], f8, tag="e", name="e")
                    # pair 25 has no plane-1 data; its stale e8 plane-1
                    # contents are finite and multiply zero mv rows
                    jsl = slice(0, 1) if t == NT2 - 1 else slice(0, 2)
                    nc.scalar.activation(
                        out=e8[:, jsl, :w],
                        in_=s2[:, jsl, :w],
                        func=mybir.ActivationFunctionType.Exp,
                        scale=0.125,  # 1/sqrt(CK)
                        bias=bias_sb[:, 0:1],
                    )
                    # denominator partial sums (pads excluded; pair 25 has
                    # only 80 real rows, all in plane 0)
                    if t == 0:
                        nc.vector.tensor_copy(
                            out=r2[:, :, :w], in_=e8[:, :, :w]
                        )
                    elif t < NT2 - 1:
                        nc.vector.tensor_add(
                            out=r2[:, :, :w], in0=r2[:, :, :w], in1=e8[:, :, :w]
                        )
                    else:
                        nc.vector.tensor_add(
                            out=r2[:NLAST, 0, :w],
                            in0=r2[:NLAST, 0, :w],
                            in1=e8[:NLAST, 0, :w],
                        )
                    first, last = t == 0, t == NT2 - 1
                    for sl in range(nsl):
                        pw = min(128, w - sl * 128)
                        nc.tensor.matmul(
                            acc[sl][:pw],
                            lhsT=e8[:, :, sl * 128:sl * 128 + pw],
                            rhs=mv8_sb[:, t, :, :],
                            start=first,
                            stop=last,
                            perf_mode=mybir.MatmulPerfMode.DoubleRow,
                        )

                if t == (NT2 // 2 - 1 if ci == LASTC else NT2 - 1):
                    # raw denominator out; host does the 256-row sum
                    nc.sync.dma_start(
                        out=l_d[:, :, ps:ps + w], in_=r2[:, :, :w]
                    )
                    last_chunk = ci == len(PCHUNKS) - 1
                    for sl in range(nsl):
                        pw = min(128, w - sl * 128)
                        o_sb = opool.tile([128, CV], f32, tag="o", name="o")
                        p0 = ps + sl * 128
                        if last_chunk:
                            # end-of-kernel latency path: split the copy
                            # across both engines and the DMA across both
                            # HWDGE queues
                            nc.vector.tensor_copy(
                                out=o_sb[:pw, :256], in_=acc[sl][:pw, :256]
                            )
                            nc.scalar.copy(
                                out=o_sb[:pw, 256:], in_=acc[sl][:pw, 256:]
                            )
                            nc.scalar.dma_start(
                                out=mem_d[p0:p0 + pw, :256],
                                in_=o_sb[:pw, :256],
                            )
                            nc.sync.dma_start(
                                out=mem_d[p0:p0 + pw, 256:],
                                in_=o_sb[:pw, 256:],
                            )
                        else:
                            nc.any.tensor_copy(
                                out=o_sb[:pw, :256], in_=acc[sl][:pw, :256]
                            )
                            nc.any.tensor_copy(
                                out=o_sb[:pw, 256:], in_=acc[sl][:pw, 256:]
                            )
                            nc.sync.dma_start(
                                out=mem_d[p0:p0 + pw, :], in_=o_sb[:pw]
                            )

    _strip_same_engine_waits(nc)
    nc.compile()
    return nc


def _strip_same_engine_waits(nc):
    """Drop redundant same-engine semaphore waits on ACT/PE compute
    instructions (see baseline notes: TRN2 instructions hold one wait;
    extra waits force serializing EventSemaphore instructions)."""
    prefixes = {
        "EngineType.Activation": "Activation_",
        "EngineType.PE": "PE_",
    }
    kinds = (mybir.InstActivation, mybir.InstMatmult, mybir.InstLdweights)
    for fn in nc.m.functions:
        for blk in fn.blocks:
            for inst in blk.instructions:
                si = getattr(inst, "sync_info", None)
                if si is None or not si.on_wait or not isinstance(inst, kinds):
                    continue
                pref = prefixes.get(str(getattr(inst, "engine", None)))
                if pref is None:
                    continue
                kept = [w for w in si.on_wait
                        if not str(getattr(w, "ant_name", "")).startswith(pref)]
                if len(kept) != len(si.on_wait):
                    si.on_wait = kept


def _get_program():
    if "nc" not in _CACHE:
        _CACHE["nc"] = _build_program()
    return _CACHE["nc"]


def _make_in_maps(mk, mv, qk):
    f8 = ml_dtypes.float8_e4m3
    mkf = np.ascontiguousarray(mk.reshape(B, CK, N))
    mvf = np.ascontiguousarray(mv.reshape(B, CV, N))
    qkf = np.ascontiguousarray(qk.reshape(B, CK, P))
    in_maps = []
    for core in range(8):
        b, half = core // 2, core % 2
        n0, n1 = half * NHALF, (half + 1) * NHALF
        mk_c = mkf[b, :, n0:n1].astype(ml_dtypes.bfloat16)   # [64, 6480]
        # zero-pad contraction dim to 128 and n to NPAD
        mk_t = np.zeros((128, NT, 128), dtype=ml_dtypes.bfloat16)
        mk_t[:CK].reshape(CK, NPAD)[:, :NHALF] = mk_c
        qk_c = np.zeros((128, P), dtype=ml_dtypes.bfloat16)
        qk_c[:CK] = qkf[b].astype(ml_dtypes.bfloat16)
        mvt = np.zeros((NPAD, CV), dtype=f8)
        mvt[:NHALF] = mvf[b, :, n0:n1].T.astype(f8)
        # DoubleRow pair layout: [128, pair, plane, CV],
        # elem (k, t, j, v) = mvT[(2t+j)*128 + k, v]
        mv8 = np.ascontiguousarray(
            mvt.reshape(NT2, 2, 128, CV).transpose(2, 0, 1, 3)
        )
        in_maps.append({"mk": np.ascontiguousarray(mk_t),
                        "qk": np.ascontiguousarray(qk_c),
                        "mv8": mv8})
    return in_maps


def _run(mk, mv, qk, qv, trace=False, **spmd_kwargs):
    nc = _get_program()
    in_maps = _make_in_maps(mk, mv, qk)
    res = run_bass_kernel_spmd(nc, in_maps, list(range(8)), trace=trace, **spmd_kwargs)
    out = np.empty((B, 2 * CV, P), dtype=np.float32)
    for b in range(B):
        m0, l0 = res.results[2 * b]["memT"], res.results[2 * b]["lsum"]
        m1, l1 = res.results[2 * b + 1]["memT"], res.results[2 * b + 1]["lsum"]
        lv = (l0 + l1).sum(axis=(0, 1))          # [P]
        out[b, :CV] = ((m0 + m1) / lv[:, None]).T
        out[b, CV:] = qv[b].reshape(CV, P)
    return out.reshape(B, 2 * CV, H, W), res


def kernel(mk, mv, qk, qv):
    out, _ = _run(np.asarray(mk), np.asarray(mv), np.asarray(qk), np.asarray(qv))
    return out
